# revision 10
# baseline (speedup 1.0000x reference)
"""LIF neuron scan (T=4) over (4, 32, 128, 56, 56) f32, sharded over 8 NeuronCores.

Per-core shard: 4 batches -> [T=4, P=128, FD=12544] f32. The time scan is
local per element; u = u*0.25 + x_t, spike = u > 1, hard reset.

v2: memory-bound design. Loads (25.7 MB/core f32) stream on the sync
HWDGE queue; spikes are written as 1-byte fp8e5 "relu encodings"
e = relu(2^20*(u-1)) emitted by the Activation engine (e > 0 <=> u > 1,
exactly: the 2^20 scale keeps every representable positive far above
the fp8e5 subnormal range), stores on the scalar HWDGE queue. The
Vector engine carries the 6 recurrence ops per chunk (3 membrane
updates + 3 resets, all f32 STT); resets write a scratch tile so the
Activation compares never block the Vector chain. Host decodes
spikes = (e > 0).
"""

import numpy as np

import concourse.bass as bass
import concourse.mybir as mybir
import concourse.tile as tile
from concourse.vector_clock import ScopedClock
from concourse.bass_utils import run_bass_kernel_spmd

T, B, C, H, W = 4, 32, 128, 56, 56
NCORES = 8
NPER = B // NCORES            # batches per core
NELEM = NPER * C * H * W      # 1,605,632 elements per core per timestep
P = 128
FD = NELEM // P               # 12544
F = 3136                      # chunk width -> 4 chunks
NCH = FD // F
DECAY = 0.25
VTH = 1.0
SCALE = float(2 ** 20)        # relu pre-scale: keeps positives >= 0.125

_MAXW = 1


def _split_drain_and_barrier(self, tick_clock, wait_clock):
    # This walrus build's CoreV3 setupSyncWait rejects >1 sem wait on a
    # TPB_CTRL (Drain) instruction; spread the tail-drain waits across
    # sequential drains on the same engine (equivalent ordering).
    drain_inst = self.nc.sync.drain()
    wait_clock.add_sem_waits(
        drain_inst.ins, ScopedClock({None: tick_clock.global_clock})
    )
    waits = list(drain_inst.ins.sync_info.on_wait)
    if len(waits) > _MAXW:
        drain_inst.ins.sync_info.on_wait = waits[:_MAXW]
        rest = waits[_MAXW:]
        while rest:
            extra = self.nc.sync.drain()
            si = extra.ins.sync_info
            if si is None:
                extra.ins.sync_info = bass._bass_rust.SyncInfo(
                    on_wait=rest[:_MAXW], on_update=[]
                )
            else:
                si.on_wait = rest[:_MAXW]
            rest = rest[_MAXW:]

    self.nc.all_engine_barrier()
    assert self.sems is not None
    popped = self.nc._tile_sem_poison_stack.pop()
    assert popped is self._sem_poison
    self.nc.clear_and_free_semaphores(list(self.sems.allocated().values()))
    self.nc.all_engine_barrier()


def _install_patch():
    if getattr(tile.TileContext, "_lif_drain_patched", False):
        return
    tile.TileContext._drain_and_barrier = _split_drain_and_barrier
    tile.TileContext._lif_drain_patched = True


def _split_waits(nc, maxw=_MAXW):
    # Generic post-pass for the same walrus limitation: any instruction
    # carrying more than `maxw` sem waits gets the excess peeled onto
    # standalone NOPs inserted immediately before it on the same engine --
    # the engine stalls at the NOPs instead, identical blocking semantics.
    k = 0
    for fn in nc.m.functions:
        for bb in fn.blocks:
            out = []
            for ins in bb.instructions:
                si = getattr(ins, "sync_info", None)
                if si is not None and len(si.on_wait) > maxw:
                    waits = list(si.on_wait)
                    for w in waits[:-maxw] if maxw else waits:
                        k += 1
                        out.append(
                            mybir.InstNoOp(
                                name=f"splitw_{k}_{ins.name}",
                                engine=ins.engine,
                                bass_nofuse=True,
                                sync_info=mybir.SyncInfo(
                                    on_wait=[w], on_update=[]
                                ),
                            )
                        )
                    si.on_wait = waits[-maxw:] if maxw else []
                out.append(ins)
            bb.instructions = out


def _build(f=F, bufs=2):
    _install_patch()
    nch = FD // f
    nc = bass.Bass()
    x = nc.dram_tensor("x", [T, P, FD], mybir.dt.float32, kind="ExternalInput")
    y = nc.dram_tensor("y", [T, P, FD], mybir.dt.float8e5, kind="ExternalOutput")
    f32 = mybir.dt.float32
    fp8 = mybir.dt.float8e5
    LE = mybir.AluOpType.is_le
    MUL, ADD = mybir.AluOpType.mult, mybir.AluOpType.add
    RELU = mybir.ActivationFunctionType.Relu

    with tile.TileContext(nc) as tc:
        # x tiles get one extra buffer of prefetch depth (3-deep) so the
        # load stream can run a full chunk ahead of the Vector engine;
        # s/r tiles stay 2-deep to fit SBUF (147 + 49 KB < 208 KB).
        with tc.tile_pool(name="px", bufs=bufs + 1) as poolx, \
             tc.tile_pool(name="p", bufs=bufs) as pool:
            neg = pool.tile([P, 1], f32, tag="neg", name="neg")
            nc.gpsimd.memset(neg[:], -SCALE)
            for g in range(nch):
                sl = bass.ts(g, f)
                xt = [
                    poolx.tile([P, f], f32, tag=f"x{t}", name=f"x{t}_{g}")
                    for t in range(T)
                ]
                st = [
                    pool.tile([P, f], fp8, tag=f"s{t}", name=f"s{t}_{g}")
                    for t in range(T)
                ]
                rt = pool.tile([P, f], f32, tag="r", name=f"r_{g}")
                for t in range(T):
                    nc.sync.dma_start(xt[t][:], x[t, :, sl])
                for t in range(T):
                    if t > 0:
                        # u_t = 0.25*r_{t-1} + x_t  (in place on x_t)
                        nc.vector.scalar_tensor_tensor(
                            xt[t][:], rt[:], DECAY, xt[t][:], MUL, ADD
                        )
                    # spike encoding: e = relu(2^20*u - 2^20); e>0 <=> u>1
                    nc.scalar.activation(
                        st[t][:], xt[t][:], RELU, bias=neg[:], scale=SCALE
                    )
                    if t < T - 1:
                        # hard reset into scratch: r = (u <= 1) * u
                        nc.vector.scalar_tensor_tensor(
                            rt[:], xt[t][:], VTH, xt[t][:], LE, MUL
                        )
                    nc.scalar.dma_start(y[t, :, sl], st[t][:])
    _split_waits(nc)
    return nc


_cache = {}


def _launch(shards, **kw):
    if "nc" not in _cache:
        _cache["nc"] = _build()
    return run_bass_kernel_spmd(
        _cache["nc"],
        [{"x": s} for s in shards],
        core_ids=list(range(NCORES)),
        **kw,
    )


def kernel(x, _launch_kw=None):
    x = np.ascontiguousarray(np.asarray(x, dtype=np.float32))
    assert x.shape == (T, B, C, H, W), x.shape
    shards = [
        np.ascontiguousarray(x[:, i * NPER : (i + 1) * NPER]).reshape(T, P, FD)
        for i in range(NCORES)
    ]
    res = _launch(shards, **(_launch_kw or {}))
    _cache["last_results"] = res
    outs = [
        (np.asarray(r["y"]).astype(np.float32) > 0)
        .astype(np.float32)
        .reshape(T, NPER, C, H, W)
        for r in res.results
    ]
    return np.concatenate(outs, axis=1)
